# revision 1
# baseline (speedup 1.0000x reference)
"""Trainium2 Bass kernel for nn_BfpQuantizer: block-floating-point
quantizer (qtorch-style float_quantize to 8-exp/7-man float == bf16 RNE,
then 8-wide shared-exponent block quantize, wl=8).

Contract: kernel(x) takes the FULL fp32 input (8, 2048, 4096) and returns
the FULL output, bit-exact with the exact-math semantics of the reference:
  fq  = bf16_rne(x)                       (== float_quantize(x, 8, 7))
  M   = max |fq| over each block of 8 (last axis)
  e   = floor(log2(M)); scale = 2^(e-6)
  out = clip(round_rne(fq/scale), -127, 127) * scale

Sharding: fully data-parallel -- batch dim 8 maps 1:1 onto the 8
NeuronCores; no cross-device communication.

Per-core pipeline (one tile = 128 partitions x 2048 fp32 elements, all
HBM DMAs are single contiguous runs):
  ACT : fq  = bf16(x)        (copy, RNE)       -- contiguous
        afq = bf16(|x|)      (Abs activation)  -- contiguous
        y   = fp32(obf)      (copy, exact)     -- contiguous
  DVE : M via 3-op max tree over afq (blocks along free axis)
        per-block scale/inv bits in int16 on the bf16 bit pattern:
          tb   = (bits(M) >> 7) << 7          biased-exponent field
          invb = 33280 - tb  == bits of 2^(6-e)   [saturation-safe form:
                 (~(tb - 16640)) + 16641 -- the HW int16 ALU saturates]
          sclb = tb - 768    == bits of 2^(e-6)
        (per-block tensors are built pair-duplicated [P, G, 2] so the two
         multiplies read them through an innermost-contiguous broadcast AP
         [g][0,4][1,2], which keeps the DVE in its 2x perf mode)
        p   = fq * inv               (exact in bf16)
        pc  = clip(p, +-127.25)      (folds the +-127.5 -> +-128 case into
                                      the later clip at +-127; 127.25 is
                                      exactly halfway between bf16 values
                                      so no other p is affected)
        r   = (pc + 1.5*2^23) - 1.5*2^23   (fp32-ALU RNE round-to-int)
        obf = r * scl                (exact in bf16)
No collectives, no transposes, no broadcast DMA traffic.
"""
import sys

sys.path.insert(0, "/opt/trn_rl_repo")

import numpy as np

import concourse.bass as bass
import concourse.tile as tile
from concourse import mybir

MAGIC = 12582912.0  # 1.5 * 2**23
N_CORES = 8
ROWS, COLS = 2048, 4096  # per-core shard (full input is (8, 2048, 4096))


def _fix_waits(nc):
    """walrus in this container encodes at most 1 sync wait per
    instruction (2 for InstEventSemaphore); Tile attaches more. Hoist the
    excess waits onto standalone NoOps just before the instruction."""
    for blk in nc.m.functions[0].blocks:
        new = []
        for inst in blk.instructions:
            si = inst.sync_info
            cap = 2 if isinstance(inst, mybir.InstEventSemaphore) else 1
            if si is not None and si.on_wait and len(si.on_wait) > cap:
                waits = list(si.on_wait)
                excess, keep = waits[:-cap], waits[-cap:]
                for k, w in enumerate(excess):
                    new.append(mybir.InstNoOp(
                        name=f"{inst.name}-hw{k}",
                        engine=inst.engine,
                        sync_info=mybir.SyncInfo(on_wait=[w], on_update=[]),
                    ))
                si.on_wait = keep
            new.append(inst)
        blk.instructions = new
    return nc


def build_nc(rows=ROWS, cols=COLS, tile_free=2048, bufs=3):
    P = 128
    TF = tile_free
    G = TF // 8
    ntiles = rows * cols // (P * TF)
    assert ntiles * P * TF == rows * cols
    A = mybir.AluOpType

    nc = bass.Bass()
    x = nc.dram_tensor("x", [rows, cols], mybir.dt.float32, kind="ExternalInput")
    y = nc.dram_tensor("y", [rows, cols], mybir.dt.float32, kind="ExternalOutput")
    xv = x.rearrange("r c -> (r c)").rearrange("(t p f) -> t p f", p=P, f=TF)
    yv = y.rearrange("r c -> (r c)").rearrange("(t p f) -> t p f", p=P, f=TF)

    with tile.TileContext(nc) as tc:
        with tc.tile_pool(name="pool", bufs=bufs) as pool:
            for t in range(ntiles):
                xt = pool.tile([P, TF], mybir.dt.float32, tag="xt")
                nc.sync.dma_start(out=xt, in_=xv[t])
                fq = pool.tile([P, G, 8], mybir.dt.bfloat16, tag="fq")
                nc.scalar.copy(fq.rearrange("p g b -> p (g b)"), xt)
                afq = pool.tile([P, G, 8], mybir.dt.bfloat16, tag="afq")
                nc.scalar.activation(afq.rearrange("p g b -> p (g b)"), xt,
                                     mybir.ActivationFunctionType.Abs)
                s1 = pool.tile([P, G, 4], mybir.dt.bfloat16, tag="s1")
                nc.vector.tensor_tensor(s1, afq[:, :, 0:4], afq[:, :, 4:8], A.max)
                s2 = pool.tile([P, G, 2], mybir.dt.bfloat16, tag="s2")
                nc.vector.tensor_tensor(s2, s1[:, :, 0:2], s1[:, :, 2:4], A.max)
                M2 = pool.tile([P, G, 2], mybir.dt.bfloat16, tag="M2")
                nc.vector.tensor_tensor(M2[:, :, 0], s2[:, :, 0], s2[:, :, 1], A.max)
                nc.vector.tensor_tensor(M2[:, :, 1], s2[:, :, 0], s2[:, :, 1], A.max)
                M2f = M2.rearrange("p g b -> p (g b)")
                tb = pool.tile([P, G, 2], mybir.dt.int16, tag="tb")
                tbf = tb.rearrange("p g b -> p (g b)")
                nc.vector.tensor_scalar(tbf, M2f.bitcast(mybir.dt.int16), 7, 7,
                                        A.logical_shift_right, A.logical_shift_left)
                t2 = pool.tile([P, G, 2], mybir.dt.int16, tag="t2")
                t2f = t2.rearrange("p g b -> p (g b)")
                nc.vector.tensor_scalar(t2f, tbf, 16640, None, A.subtract)
                t3 = pool.tile([P, G, 2], mybir.dt.int16, tag="t3")
                t3f = t3.rearrange("p g b -> p (g b)")
                nc.vector.tensor_scalar(t3f, t2f, 0, None, A.bitwise_not)
                invb = pool.tile([P, G, 2], mybir.dt.int16, tag="invb")
                nc.vector.tensor_scalar(invb.rearrange("p g b -> p (g b)"), t3f,
                                        16641, None, A.add)
                sclb = pool.tile([P, G, 2], mybir.dt.int16, tag="sclb")
                nc.vector.tensor_scalar(sclb.rearrange("p g b -> p (g b)"), tbf,
                                        768, None, A.subtract)
                inv2 = invb.bitcast(mybir.dt.bfloat16)
                scl2 = sclb.bitcast(mybir.dt.bfloat16)
                inv_b = inv2.unsqueeze(2).broadcast_to((P, G, 4, 2))
                scl_b = scl2.unsqueeze(2).broadcast_to((P, G, 4, 2))
                fq4 = fq.rearrange("p g (c b) -> p g c b", b=2)
                p_t = pool.tile([P, G, 4, 2], mybir.dt.bfloat16, tag="p")
                nc.vector.tensor_tensor(p_t, fq4, inv_b, A.mult)
                pf = p_t.rearrange("p g c b -> p (g c b)")
                pc = pool.tile([P, TF], mybir.dt.bfloat16, tag="pc")
                nc.vector.tensor_scalar(pc, pf, 127.25, -127.25, A.min, A.max)
                r = pool.tile([P, TF], mybir.dt.bfloat16, tag="r")
                nc.vector.tensor_scalar(r, pc, MAGIC, MAGIC, A.add, A.subtract)
                obf = pool.tile([P, G, 4, 2], mybir.dt.bfloat16, tag="obf")
                nc.vector.tensor_tensor(obf,
                                        r.rearrange("p (g c b) -> p g c b", g=G, b=2),
                                        scl_b, A.mult)
                yt = pool.tile([P, TF], mybir.dt.float32, tag="yt")
                nc.scalar.copy(yt, obf.rearrange("p g c b -> p (g c b)"))
                nc.sync.dma_start(out=yv[t], in_=yt)
    _fix_waits(nc)
    return nc


_CACHED_NC = None


def _get_nc():
    global _CACHED_NC
    if _CACHED_NC is None:
        _CACHED_NC = build_nc()
    return _CACHED_NC


def kernel(x: np.ndarray) -> np.ndarray:
    """Full-input entry point: x (8, 2048, 4096) fp32 -> same-shape fp32."""
    from concourse.bass_utils import run_bass_kernel_spmd

    x = np.ascontiguousarray(np.asarray(x, dtype=np.float32))
    assert x.shape == (N_CORES, ROWS, COLS), x.shape
    nc = _get_nc()
    in_maps = [{"x": x[i]} for i in range(N_CORES)]
    res = run_bass_kernel_spmd(nc, in_maps, list(range(N_CORES)))
    out = np.stack([res.results[i]["y"] for i in range(N_CORES)])
    return out.astype(np.float32, copy=False)

